# revision 30
# baseline (speedup 1.0000x reference)
"""DTW layer (short kernel) Trainium2 Bass kernel.

Problem: x (B=8, C=8, L=4096) f32, kernels (F=32, K=10) f32.
For each (b, c, f, w): DTW cost between kernels[f] (len 10) and window
x[b, c, 5w : 5w+20], for w in [0, 815). Output (B, C*F, 815) f32.

Sharding: data-parallel over batch — core b computes batch b entirely
(C*F = 256 (c,f) combos = 2 partition chunks of 128).

Algorithm (per core): the DTW row recurrence
    row_i[j] = D[i,j] + min(row_i[j-1], row_{i-1}[j], row_{i-1}[j-1])
is computed for 128 (c,f) combos at once (partition dim) and a chunk of
windows laid out along the free dim as [w, 21] segments (1 separator +
20 cells).  Per row:
  - ACT computes local costs D[w, 1+j] = (x[5w+j] - k_i)^2 via
    activation(Square, bias=-k_i) with an overlapping strided input AP.
  - GPSIMD (or DVE) computes m[t] = min(S_prev[t], S_prev[t-1]).
  - DVE tensor_tensor_scan: state = min(m[t], state) + D[t] computes the
    whole row for all windows in one instruction.  A BIG value in the
    separator column of D forces the carry to BIG between windows, which
    the min against m (= prev row values) then discards — resetting the
    recurrence at each window boundary.

Raw bass (no Tile framework): this toolchain's walrus codegen allows at
most 2 embedded sync-waits per instruction and rejects Tile's tail
drain, so engines are programmed directly with standalone wait_ge
instructions and per-engine semaphores.
"""

from contextlib import ExitStack

import numpy as np

import concourse.bass as bass
import concourse.mybir as mybir
from concourse.bass_utils import run_bass_kernel_spmd

# Problem constants (hardcoded per harness contract)
B, C, L = 8, 8, 4096
F, K = 32, 10
PROC, STEP = 20, 5
NW = 815          # windows actually computed == chan_outlen
SEG = PROC + 1    # 1 separator + 20 cells
NWC = 136         # windows per chunk; 6 chunks = 816 >= 815
NCHUNK = 6
TFREE = NWC * SEG # 2856 scan length
BIG = 1e30
SLOTS = 2
UNITS = [(cc, wc) for cc in range(2) for wc in range(NCHUNK)]

F32 = mybir.dt.float32
F16 = mybir.dt.float16


def _build_nc(reps: int = 1, gp_m: bool = False, dt16: bool = False,
              small_m: bool = False, small_scan: bool = False,
              small_act: bool = False, ileave: bool = True) -> bass.Bass:
    """gp_m: run the shifted-min on GPSIMD (off DVE's critical path).
    dt16: keep state/cost tiles in bf16 (DVE 2x mode candidates).
    small_*: shrink one op class to 4 elements (timing attribution).
    reps > 1 replicates the schedule (slope-based timing)."""
    # detect_race_conditions=False: CoreSim's detector does not model
    # same-engine program order, which this kernel relies on throughout.
    nc = bass.Bass("TRN2", debug=False, detect_race_conditions=False)
    x_d = nc.dram_tensor("x", [C, L], F32, kind="ExternalInput").ap()
    k_d = nc.dram_tensor("negk", [F, K], F32, kind="ExternalInput").ap()
    out_d = nc.dram_tensor("out", [C * F, NWC * NCHUNK], F32,
                           kind="ExternalOutput").ap()

    UNITS_R = UNITS * reps
    SDT = F16 if dt16 else F32
    big = 30000.0 if dt16 else BIG

    # --- semaphore bookkeeping (python-side op counts) ---
    # DVE emission order as an explicit list: 8 init memsets, then per
    # unit scan0 + (m, scan) x 9 (m omitted under gp_m).  With ileave,
    # unit pairs (2k, 2k+1) are interleaved row-by-row so one unit's
    # cross-engine latency is hidden behind the partner's ops.
    dve_ops = []  # ("m"|"scan", u, i)
    nu = len(UNITS) * reps
    if ileave:
        for base in range(0, nu, 2):
            pair = [base] + ([base + 1] if base + 1 < nu else [])
            for i in range(K):
                for u in pair:
                    if i > 1 and not gp_m:
                        dve_ops.append(("m", u, i))
                    dve_ops.append(("scan", u, i))
    else:
        for u in range(nu):
            for i in range(K):
                if i > 0 and not gp_m:
                    dve_ops.append(("m", u, i))
                dve_ops.append(("scan", u, i))
    _scan_pos = {(u, i): 8 + n + 1
                 for n, (kind, u, i) in enumerate(dve_ops)
                 if kind == "scan"}

    def dve_through_scan(u, i):
        return _scan_pos[(u, i)]

    def gp_through_m(u, i):  # 9 m-ops per unit, i in 1..9
        return 9 * u + i

    # ACT order: pair-interleaved to match the DVE order: per pair,
    # squares (u0,i),(u1,i) for each i, then both extract copies.
    act_ops = []  # ("sq"|"cp", u, i)
    if ileave:
        for base in range(0, nu, 2):
            pair = [base] + ([base + 1] if base + 1 < nu else [])
            for i in range(K):
                for u in pair:
                    act_ops.append(("sq", u, i))
                if i == 1:
                    for u in pair:
                        act_ops.append(("m1a", u, 0))
                        act_ops.append(("m1b", u, 0))
            for u in pair:
                act_ops.append(("cp", u, 0))
    else:
        for u in range(nu):
            for i in range(K):
                act_ops.append(("sq", u, i))
            act_ops.append(("cp", u, 0))
    _sq_pos = {(u, i): n + 1 for n, (kind, u, i) in enumerate(act_ops)
               if kind == "sq"}
    _m1_pos = {u: n + 1 for n, (kind, u, i) in enumerate(act_ops)
               if kind == "m1b"}
    _cp_pos = {u: n + 1 for n, (kind, u, i) in enumerate(act_ops)
               if kind == "cp"}

    def act_through_square(u, i):
        return _sq_pos[(u, i)]

    def act_through_copy(u):
        return _cp_pos[u]

    def dma_through_out(u):  # X1 init DMA then one out-DMA per unit
        return 16 * (2 + u)

    with ExitStack() as ctx:
        sb = lambda shape, name, dt: ctx.enter_context(
            nc.sbuf_tensor(name, shape, dt))
        X = [sb([128, L], f"Xt{cc}", F32) for cc in range(2)]
        negK = sb([128, K], "negKt", F32)
        m0 = sb([128, TFREE], "m0t", SDT)
        S = [[sb([128, TFREE], f"St{s}_{i}", SDT) for i in range(2)]
             for s in range(SLOTS)]
        M = [sb([128, TFREE], f"Mt{s}", SDT) for s in range(SLOTS)]
        D = [[sb([128, TFREE], f"Dt{s}_{i}", SDT) for i in range(2)]
             for s in range(SLOTS)]
        OB = [sb([128, NWC], f"OBt{s}", F32) for s in range(SLOTS)]

        dma_sem = ctx.enter_context(nc.semaphore("dma_sem"))
        dma0_sem = ctx.enter_context(nc.semaphore("dma0_sem"))
        act_sem = ctx.enter_context(nc.semaphore("act_sem"))
        dve_sem = ctx.enter_context(nc.semaphore("dve_sem"))
        gp_sem = ctx.enter_context(nc.semaphore("gp_sem"))
        block = ctx.enter_context(nc.Block())

        @block.sync
        def _(sync):
            # negK + X0 first so cc0 compute starts before X1 lands.
            # X[cc] partition p holds x[4*cc + p//32, :] (source AP
            # replicates each channel row 32x via a step-0 dim)
            ksrc = bass.AP(k_d.tensor, 0, [[0, 4], [K, F], [1, K]])
            sync.dma_start(negK.ap(), ksrc).then_inc(dma0_sem, 16)
            for cc in range(2):
                src = bass.AP(x_d.tensor, 4 * cc * L,
                              [[L, 4], [0, 32], [1, L]])
                sync.dma_start(X[cc].ap(), src).then_inc(
                    dma0_sem if cc == 0 else dma_sem, 16)
            for u, (cc, wc) in enumerate(UNITS_R):
                s = u % SLOTS
                sync.wait_ge(act_sem, act_through_copy(u))
                sync.dma_start(
                    out_d[128 * cc:128 * (cc + 1),
                          NWC * wc:NWC * (wc + 1)],
                    OB[s].ap()).then_inc(dma_sem, 16)

        def emit_m(eng, u, s, prev):
            if small_m:
                return eng.tensor_tensor(M[s].ap()[:, 1:5], prev[:, 1:5],
                                         prev[:, :4],
                                         mybir.AluOpType.min)
            return eng.tensor_tensor(M[s].ap()[:, 1:], prev[:, 1:],
                                     prev[:, :-1], mybir.AluOpType.min)

        if gp_m:
            @block.gpsimd
            def _(gpsimd):
                dve_waited = 0
                for u, (cc, wc) in enumerate(UNITS_R):
                    s = u % SLOTS
                    for i in range(1, K):
                        need = dve_through_scan(u, i - 1)
                        if need > dve_waited:
                            gpsimd.wait_ge(dve_sem, need)
                            dve_waited = need
                        emit_m(gpsimd, u, s,
                               S[s][(i - 1) % 2].ap()).then_inc(gp_sem, 1)

        @block.vector
        def _(vector):
            # init: m0 = BIG with 0 at each segment's cell j=0 (offset 1);
            # M BIG (so m[0] defined); D separator columns BIG
            vector.memset(m0.ap(), big).then_inc(dve_sem, 1)
            m0_seg = m0.ap().rearrange("p (w s) -> p w s", s=SEG)
            vector.memset(m0_seg[:, :, 1], 0.0).then_inc(dve_sem, 1)
            for s in range(SLOTS):
                vector.memset(M[s].ap(), big).then_inc(dve_sem, 1)
                for i in range(2):
                    d_seg = D[s][i].ap().rearrange("p (w s) -> p w s", s=SEG)
                    vector.memset(d_seg[:, :, 0], big).then_inc(dve_sem, 1)
            act_waited = 0
            gp_waited = 0
            for kind, u, i in dve_ops:
                s = u % SLOTS
                if kind == "m":
                    if gp_m:
                        continue
                    emit_m(vector, u, s,
                           S[s][(i - 1) % 2].ap()).then_inc(dve_sem, 1)
                    continue
                # scan: row i reads m0 (i=0) or M[s], writes S[s][i%2]
                if i == 0:
                    m_ap = m0.ap()
                else:
                    if gp_m:
                        need = gp_through_m(u, i)
                        if need > gp_waited:
                            vector.wait_ge(gp_sem, need)
                            gp_waited = need
                    elif i == 1:
                        need = _m1_pos[u]
                        if need > act_waited:
                            vector.wait_ge(act_sem, need)
                            act_waited = need
                    m_ap = M[s].ap()
                need = act_through_square(u, i)
                if need > act_waited:
                    vector.wait_ge(act_sem, need)
                    act_waited = need
                if small_scan:
                    vector.tensor_tensor_scan(
                        S[s][i % 2].ap()[:, :4], m_ap[:, :4],
                        D[s][i % 2].ap()[:, :4], float(big),
                        op0=mybir.AluOpType.min,
                        op1=mybir.AluOpType.add).then_inc(dve_sem, 1)
                else:
                    vector.tensor_tensor_scan(
                        S[s][i % 2].ap(), m_ap, D[s][i % 2].ap(),
                        float(big),
                        op0=mybir.AluOpType.min,
                        op1=mybir.AluOpType.add).then_inc(dve_sem, 1)

        @block.scalar
        def _(scalar):
            scalar.wait_ge(dma0_sem, 32)  # negK + X0
            dve_waited = 0
            dma_waited = 0
            x1_waited = False
            for kind, u, i in act_ops:
                cc, wc = UNITS_R[u]
                s = u % SLOTS
                if cc == 1 and not x1_waited:
                    scalar.wait_ge(dma_sem, 16)  # X1
                    x1_waited = True
                if kind == "sq":
                    xt = X[cc].ap()
                    win = bass.AP(xt.tensor, xt.offset + 5 * NWC * wc,
                                  [list(xt.ap[0]), [5, NWC], [1, PROC]])
                    # WAR: D[s][i%2] was last read by an earlier scan
                    if i >= 2:
                        need = dve_through_scan(u, i - 2)
                    elif u >= SLOTS:
                        need = dve_through_scan(u - SLOTS, 8 + i)
                    else:
                        need = 0
                    if need > dve_waited:
                        scalar.wait_ge(dve_sem, need)
                        dve_waited = need
                    d_seg = D[s][i % 2].ap().rearrange(
                        "p (w s) -> p w s", s=SEG)
                    if small_act:
                        scalar.activation(
                            d_seg[:, :1, 1:], win[:, :1, :],
                            mybir.ActivationFunctionType.Square,
                            bias=negK.ap()[:, i:i + 1],
                            scale=1.0).then_inc(act_sem, 1)
                    else:
                        scalar.activation(
                            d_seg[:, :, 1:], win,
                            mybir.ActivationFunctionType.Square,
                            bias=negK.ap()[:, i:i + 1],
                            scale=1.0).then_inc(act_sem, 1)
                elif kind == "m1a":
                    need = dve_through_scan(u, 0)
                    if need > dve_waited:
                        scalar.wait_ge(dve_sem, need)
                        dve_waited = need
                    scalar.copy(M[s].ap()[:, 1:],
                                S[s][0].ap()[:, :-1]).then_inc(act_sem, 1)
                elif kind == "m1b":
                    sseg = S[s][0].ap().rearrange("p (w s) -> p w s", s=SEG)
                    mseg = M[s].ap().rearrange("p (w s) -> p w s", s=SEG)
                    scalar.copy(mseg[:, :, 1],
                                sseg[:, :, 1]).then_inc(act_sem, 1)
                else:
                    # extract: cell j=19 lives at segment offset 20; final
                    # row (i=9, odd) lands in S[s][1]
                    need = dve_through_scan(u, K - 1)
                    if need > dve_waited:
                        scalar.wait_ge(dve_sem, need)
                        dve_waited = need
                    if u >= SLOTS:
                        dneed = dma_through_out(u - SLOTS)
                        if dneed > dma_waited:
                            scalar.wait_ge(dma_sem, dneed)
                            dma_waited = dneed
                    s_seg = S[s][1].ap().rearrange("p (w s) -> p w s",
                                                   s=SEG)
                    scalar.copy(OB[s].ap(), s_seg[:, :, SEG - 1]).then_inc(
                        act_sem, 1)
    return nc


_NC_CACHE = None


def kernel(x: np.ndarray, kernels: np.ndarray) -> np.ndarray:
    global _NC_CACHE
    if _NC_CACHE is None:
        _NC_CACHE = _build_nc()
    nc = _NC_CACHE
    x = np.ascontiguousarray(x, dtype=np.float32)
    negk = np.ascontiguousarray(-np.asarray(kernels, dtype=np.float32))
    in_maps = [{"x": x[b], "negk": negk} for b in range(B)]
    res = run_bass_kernel_spmd(nc, in_maps, core_ids=list(range(B)))
    out = np.stack([res.results[b]["out"] for b in range(B)], axis=0)
    return out[:, :, :NW]


# revision 32
# speedup vs baseline: 1.0039x; 1.0039x over previous
"""DTW layer (short kernel) Trainium2 Bass kernel.

Problem: x (B=8, C=8, L=4096) f32, kernels (F=32, K=10) f32.
For each (b, c, f, w): DTW cost between kernels[f] (len 10) and window
x[b, c, 5w : 5w+20], for w in [0, 815). Output (B, C*F, 815) f32.

Sharding: data-parallel over batch — core b computes batch b entirely
(C*F = 256 (c,f) combos = 2 partition chunks of 128).

Algorithm (per core): the DTW row recurrence
    row_i[j] = D[i,j] + min(row_i[j-1], row_{i-1}[j], row_{i-1}[j-1])
is computed for 128 (c,f) combos at once (partition dim) and a chunk of
windows laid out along the free dim as [w, 21] segments (1 separator +
20 cells).  Per row:
  - ACT computes local costs D[w, 1+j] = (x[5w+j] - k_i)^2 via
    activation(Square, bias=-k_i) with an overlapping strided input AP.
  - GPSIMD (or DVE) computes m[t] = min(S_prev[t], S_prev[t-1]).
  - DVE tensor_tensor_scan: state = min(m[t], state) + D[t] computes the
    whole row for all windows in one instruction.  A BIG value in the
    separator column of D forces the carry to BIG between windows, which
    the min against m (= prev row values) then discards — resetting the
    recurrence at each window boundary.

Raw bass (no Tile framework): this toolchain's walrus codegen allows at
most 2 embedded sync-waits per instruction and rejects Tile's tail
drain, so engines are programmed directly with standalone wait_ge
instructions and per-engine semaphores.
"""

from contextlib import ExitStack

import numpy as np

import concourse.bass as bass
import concourse.mybir as mybir
from concourse.bass_utils import run_bass_kernel_spmd

# Problem constants (hardcoded per harness contract)
B, C, L = 8, 8, 4096
F, K = 32, 10
PROC, STEP = 20, 5
NW = 815          # windows actually computed == chan_outlen
SEG = PROC + 1    # 1 separator + 20 cells
NWC = 102         # windows per chunk; 8 chunks = 816 >= 815
NCHUNK = 8
TFREE = NWC * SEG # 2856 scan length
BIG = 1e30
SLOTS = 2
UNITS = [(cc, wc) for cc in range(2) for wc in range(NCHUNK)]

F32 = mybir.dt.float32
F16 = mybir.dt.float16


def _build_nc(reps: int = 1, gp_m: bool = False, dt16: bool = False,
              small_m: bool = False, small_scan: bool = False,
              small_act: bool = False, ileave: bool = True) -> bass.Bass:
    """gp_m: run the shifted-min on GPSIMD (off DVE's critical path).
    dt16: keep state/cost tiles in bf16 (DVE 2x mode candidates).
    small_*: shrink one op class to 4 elements (timing attribution).
    reps > 1 replicates the schedule (slope-based timing)."""
    # detect_race_conditions=False: CoreSim's detector does not model
    # same-engine program order, which this kernel relies on throughout.
    nc = bass.Bass("TRN2", debug=False, detect_race_conditions=False)
    x_d = nc.dram_tensor("x", [C, L], F32, kind="ExternalInput").ap()
    k_d = nc.dram_tensor("negk", [F, K], F32, kind="ExternalInput").ap()
    out_d = nc.dram_tensor("out", [C * F, NWC * NCHUNK], F32,
                           kind="ExternalOutput").ap()

    UNITS_R = UNITS * reps
    SDT = F16 if dt16 else F32
    big = 30000.0 if dt16 else BIG

    # --- semaphore bookkeeping (python-side op counts) ---
    # DVE emission order as an explicit list: 8 init memsets, then per
    # unit scan0 + (m, scan) x 9 (m omitted under gp_m).  With ileave,
    # unit pairs (2k, 2k+1) are interleaved row-by-row so one unit's
    # cross-engine latency is hidden behind the partner's ops.
    dve_ops = []  # ("m"|"scan", u, i)
    nu = len(UNITS) * reps
    if ileave:
        for base in range(0, nu, 2):
            pair = [base] + ([base + 1] if base + 1 < nu else [])
            for i in range(K):
                for u in pair:
                    if i > 0 and not gp_m:
                        dve_ops.append(("m", u, i))
                    dve_ops.append(("scan", u, i))
    else:
        for u in range(nu):
            for i in range(K):
                if i > 0 and not gp_m:
                    dve_ops.append(("m", u, i))
                dve_ops.append(("scan", u, i))
    _scan_pos = {(u, i): 8 + n + 1
                 for n, (kind, u, i) in enumerate(dve_ops)
                 if kind == "scan"}

    def dve_through_scan(u, i):
        return _scan_pos[(u, i)]

    def gp_through_m(u, i):  # 9 m-ops per unit, i in 1..9
        return 9 * u + i

    # ACT order: pair-interleaved to match the DVE order: per pair,
    # squares (u0,i),(u1,i) for each i, then both extract copies.
    act_ops = []  # ("sq"|"cp", u, i)
    if ileave:
        for base in range(0, nu, 2):
            pair = [base] + ([base + 1] if base + 1 < nu else [])
            for i in range(K):
                for u in pair:
                    act_ops.append(("sq", u, i))
            for u in pair:
                act_ops.append(("cp", u, 0))
    else:
        for u in range(nu):
            for i in range(K):
                act_ops.append(("sq", u, i))
            act_ops.append(("cp", u, 0))
    _sq_pos = {(u, i): n + 1 for n, (kind, u, i) in enumerate(act_ops)
               if kind == "sq"}
    _cp_pos = {u: n + 1 for n, (kind, u, i) in enumerate(act_ops)
               if kind == "cp"}

    def act_through_square(u, i):
        return _sq_pos[(u, i)]

    def act_through_copy(u):
        return _cp_pos[u]

    def dma_through_out(u):  # X1 init DMA then one out-DMA per unit
        return 16 * (2 + u)

    with ExitStack() as ctx:
        sb = lambda shape, name, dt: ctx.enter_context(
            nc.sbuf_tensor(name, shape, dt))
        X = [sb([128, L], f"Xt{cc}", F32) for cc in range(2)]
        negK = sb([128, K], "negKt", F32)
        m0 = sb([128, TFREE], "m0t", SDT)
        S = [[sb([128, TFREE], f"St{s}_{i}", SDT) for i in range(2)]
             for s in range(SLOTS)]
        M = [sb([128, TFREE], f"Mt{s}", SDT) for s in range(SLOTS)]
        D = [[sb([128, TFREE], f"Dt{s}_{i}", SDT) for i in range(2)]
             for s in range(SLOTS)]
        OB = [sb([128, NWC], f"OBt{s}", F32) for s in range(SLOTS)]

        dma_sem = ctx.enter_context(nc.semaphore("dma_sem"))
        dma0_sem = ctx.enter_context(nc.semaphore("dma0_sem"))
        act_sem = ctx.enter_context(nc.semaphore("act_sem"))
        dve_sem = ctx.enter_context(nc.semaphore("dve_sem"))
        gp_sem = ctx.enter_context(nc.semaphore("gp_sem"))
        block = ctx.enter_context(nc.Block())

        @block.sync
        def _(sync):
            # negK + X0 first so cc0 compute starts before X1 lands.
            # X[cc] partition p holds x[4*cc + p//32, :] (source AP
            # replicates each channel row 32x via a step-0 dim)
            ksrc = bass.AP(k_d.tensor, 0, [[0, 4], [K, F], [1, K]])
            sync.dma_start(negK.ap(), ksrc).then_inc(dma0_sem, 16)
            for cc in range(2):
                src = bass.AP(x_d.tensor, 4 * cc * L,
                              [[L, 4], [0, 32], [1, L]])
                sync.dma_start(X[cc].ap(), src).then_inc(
                    dma0_sem if cc == 0 else dma_sem, 16)
            for u, (cc, wc) in enumerate(UNITS_R):
                s = u % SLOTS
                sync.wait_ge(act_sem, act_through_copy(u))
                sync.dma_start(
                    out_d[128 * cc:128 * (cc + 1),
                          NWC * wc:NWC * (wc + 1)],
                    OB[s].ap()).then_inc(dma_sem, 16)

        def emit_m(eng, u, s, prev):
            if small_m:
                return eng.tensor_tensor(M[s].ap()[:, 1:5], prev[:, 1:5],
                                         prev[:, :4],
                                         mybir.AluOpType.min)
            return eng.tensor_tensor(M[s].ap()[:, 1:], prev[:, 1:],
                                     prev[:, :-1], mybir.AluOpType.min)

        if gp_m:
            @block.gpsimd
            def _(gpsimd):
                dve_waited = 0
                for u, (cc, wc) in enumerate(UNITS_R):
                    s = u % SLOTS
                    for i in range(1, K):
                        need = dve_through_scan(u, i - 1)
                        if need > dve_waited:
                            gpsimd.wait_ge(dve_sem, need)
                            dve_waited = need
                        emit_m(gpsimd, u, s,
                               S[s][(i - 1) % 2].ap()).then_inc(gp_sem, 1)

        @block.vector
        def _(vector):
            # init: m0 = BIG with 0 at each segment's cell j=0 (offset 1);
            # M BIG (so m[0] defined); D separator columns BIG
            vector.memset(m0.ap(), big).then_inc(dve_sem, 1)
            m0_seg = m0.ap().rearrange("p (w s) -> p w s", s=SEG)
            vector.memset(m0_seg[:, :, 1], 0.0).then_inc(dve_sem, 1)
            for s in range(SLOTS):
                vector.memset(M[s].ap(), big).then_inc(dve_sem, 1)
                for i in range(2):
                    d_seg = D[s][i].ap().rearrange("p (w s) -> p w s", s=SEG)
                    vector.memset(d_seg[:, :, 0], big).then_inc(dve_sem, 1)
            act_waited = 0
            gp_waited = 0
            for kind, u, i in dve_ops:
                s = u % SLOTS
                if kind == "m":
                    if gp_m:
                        continue
                    emit_m(vector, u, s,
                           S[s][(i - 1) % 2].ap()).then_inc(dve_sem, 1)
                    continue
                # scan: row i reads m0 (i=0) or M[s], writes S[s][i%2]
                if i == 0:
                    m_ap = m0.ap()
                else:
                    if gp_m:
                        need = gp_through_m(u, i)
                        if need > gp_waited:
                            vector.wait_ge(gp_sem, need)
                            gp_waited = need
                    m_ap = M[s].ap()
                need = act_through_square(u, i)
                if need > act_waited:
                    vector.wait_ge(act_sem, need)
                    act_waited = need
                if small_scan:
                    vector.tensor_tensor_scan(
                        S[s][i % 2].ap()[:, :4], m_ap[:, :4],
                        D[s][i % 2].ap()[:, :4], float(big),
                        op0=mybir.AluOpType.min,
                        op1=mybir.AluOpType.add).then_inc(dve_sem, 1)
                else:
                    vector.tensor_tensor_scan(
                        S[s][i % 2].ap(), m_ap, D[s][i % 2].ap(),
                        float(big),
                        op0=mybir.AluOpType.min,
                        op1=mybir.AluOpType.add).then_inc(dve_sem, 1)

        @block.scalar
        def _(scalar):
            scalar.wait_ge(dma0_sem, 32)  # negK + X0
            dve_waited = 0
            dma_waited = 0
            x1_waited = False
            for kind, u, i in act_ops:
                cc, wc = UNITS_R[u]
                s = u % SLOTS
                if cc == 1 and not x1_waited:
                    scalar.wait_ge(dma_sem, 16)  # X1
                    x1_waited = True
                if kind == "sq":
                    xt = X[cc].ap()
                    win = bass.AP(xt.tensor, xt.offset + 5 * NWC * wc,
                                  [list(xt.ap[0]), [5, NWC], [1, PROC]])
                    # WAR: D[s][i%2] was last read by an earlier scan
                    if i >= 2:
                        need = dve_through_scan(u, i - 2)
                    elif u >= SLOTS:
                        need = dve_through_scan(u - SLOTS, 8 + i)
                    else:
                        need = 0
                    if need > dve_waited:
                        scalar.wait_ge(dve_sem, need)
                        dve_waited = need
                    d_seg = D[s][i % 2].ap().rearrange(
                        "p (w s) -> p w s", s=SEG)
                    if small_act:
                        scalar.activation(
                            d_seg[:, :1, 1:], win[:, :1, :],
                            mybir.ActivationFunctionType.Square,
                            bias=negK.ap()[:, i:i + 1],
                            scale=1.0).then_inc(act_sem, 1)
                    else:
                        scalar.activation(
                            d_seg[:, :, 1:], win,
                            mybir.ActivationFunctionType.Square,
                            bias=negK.ap()[:, i:i + 1],
                            scale=1.0).then_inc(act_sem, 1)
                else:
                    # extract: cell j=19 lives at segment offset 20; final
                    # row (i=9, odd) lands in S[s][1]
                    need = dve_through_scan(u, K - 1)
                    if need > dve_waited:
                        scalar.wait_ge(dve_sem, need)
                        dve_waited = need
                    if u >= SLOTS:
                        dneed = dma_through_out(u - SLOTS)
                        if dneed > dma_waited:
                            scalar.wait_ge(dma_sem, dneed)
                            dma_waited = dneed
                    s_seg = S[s][1].ap().rearrange("p (w s) -> p w s",
                                                   s=SEG)
                    scalar.copy(OB[s].ap(), s_seg[:, :, SEG - 1]).then_inc(
                        act_sem, 1)
    return nc


_NC_CACHE = None


def kernel(x: np.ndarray, kernels: np.ndarray) -> np.ndarray:
    global _NC_CACHE
    if _NC_CACHE is None:
        _NC_CACHE = _build_nc()
    nc = _NC_CACHE
    x = np.ascontiguousarray(x, dtype=np.float32)
    negk = np.ascontiguousarray(-np.asarray(kernels, dtype=np.float32))
    in_maps = [{"x": x[b], "negk": negk} for b in range(B)]
    res = run_bass_kernel_spmd(nc, in_maps, core_ids=list(range(B)))
    out = np.stack([res.results[b]["out"] for b in range(B)], axis=0)
    return out[:, :, :NW]


# revision 33
# speedup vs baseline: 1.0193x; 1.0153x over previous
"""DTW layer (short kernel) Trainium2 Bass kernel.

Problem: x (B=8, C=8, L=4096) f32, kernels (F=32, K=10) f32.
For each (b, c, f, w): DTW cost between kernels[f] (len 10) and window
x[b, c, 5w : 5w+20], for w in [0, 815). Output (B, C*F, 815) f32.

Sharding: data-parallel over batch — core b computes batch b entirely
(C*F = 256 (c,f) combos = 2 partition chunks of 128).

Algorithm (per core): the DTW row recurrence
    row_i[j] = D[i,j] + min(row_i[j-1], row_{i-1}[j], row_{i-1}[j-1])
is computed for 128 (c,f) combos at once (partition dim) and a chunk of
windows laid out along the free dim as [w, 21] segments (1 separator +
20 cells).  Per row:
  - ACT computes local costs D[w, 1+j] = (x[5w+j] - k_i)^2 via
    activation(Square, bias=-k_i) with an overlapping strided input AP.
  - GPSIMD (or DVE) computes m[t] = min(S_prev[t], S_prev[t-1]).
  - DVE tensor_tensor_scan: state = min(m[t], state) + D[t] computes the
    whole row for all windows in one instruction.  A BIG value in the
    separator column of D forces the carry to BIG between windows, which
    the min against m (= prev row values) then discards — resetting the
    recurrence at each window boundary.

Raw bass (no Tile framework): this toolchain's walrus codegen allows at
most 2 embedded sync-waits per instruction and rejects Tile's tail
drain, so engines are programmed directly with standalone wait_ge
instructions and per-engine semaphores.
"""

from contextlib import ExitStack

import numpy as np

import concourse.bass as bass
import concourse.mybir as mybir
from concourse.bass_utils import run_bass_kernel_spmd

# Problem constants (hardcoded per harness contract)
B, C, L = 8, 8, 4096
F, K = 32, 10
PROC, STEP = 20, 5
NW = 815          # windows actually computed == chan_outlen
SEG = PROC + 1    # 1 separator + 20 cells
NWC = 136         # windows per chunk; 6 chunks = 816 >= 815
NCHUNK = 6
TFREE = NWC * SEG # 2856 scan length
BIG = 1e30
SLOTS = 2
UNITS = [(cc, wc) for cc in range(2) for wc in range(NCHUNK)]

F32 = mybir.dt.float32
F16 = mybir.dt.float16


def _build_nc(reps: int = 1, gp_m: bool = False, dt16: bool = False,
              small_m: bool = False, small_scan: bool = False,
              small_act: bool = False, ileave: bool = True) -> bass.Bass:
    """gp_m: run the shifted-min on GPSIMD (off DVE's critical path).
    dt16: keep state/cost tiles in bf16 (DVE 2x mode candidates).
    small_*: shrink one op class to 4 elements (timing attribution).
    reps > 1 replicates the schedule (slope-based timing)."""
    # detect_race_conditions=False: CoreSim's detector does not model
    # same-engine program order, which this kernel relies on throughout.
    nc = bass.Bass("TRN2", debug=False, detect_race_conditions=False)
    x_d = nc.dram_tensor("x", [C, L], F32, kind="ExternalInput").ap()
    k_d = nc.dram_tensor("negk", [F, K], F32, kind="ExternalInput").ap()
    out_d = nc.dram_tensor("out", [C * F, NWC * NCHUNK], F32,
                           kind="ExternalOutput").ap()

    UNITS_R = UNITS * reps
    SDT = F16 if dt16 else F32
    big = 30000.0 if dt16 else BIG

    # --- semaphore bookkeeping (python-side op counts) ---
    # DVE emission order as an explicit list: 8 init memsets, then per
    # unit scan0 + (m, scan) x 9 (m omitted under gp_m).  With ileave,
    # unit pairs (2k, 2k+1) are interleaved row-by-row so one unit's
    # cross-engine latency is hidden behind the partner's ops.
    dve_ops = []  # ("m"|"scan", u, i)
    nu = len(UNITS) * reps
    if ileave:
        for base in range(0, nu, 2):
            pair = [base] + ([base + 1] if base + 1 < nu else [])
            for i in range(K):
                for u in pair:
                    if i > 0 and not gp_m:
                        dve_ops.append(("m", u, i))
                    dve_ops.append(("scan", u, i))
    else:
        for u in range(nu):
            for i in range(K):
                if i > 0 and not gp_m:
                    dve_ops.append(("m", u, i))
                dve_ops.append(("scan", u, i))
    _scan_pos = {(u, i): 8 + n + 1
                 for n, (kind, u, i) in enumerate(dve_ops)
                 if kind == "scan"}

    def dve_through_scan(u, i):
        return _scan_pos[(u, i)]

    def gp_through_m(u, i):  # 9 m-ops per unit, i in 1..9
        return 9 * u + i

    # ACT order: pair-interleaved to match the DVE order: per pair,
    # squares (u0,i),(u1,i) for each i, then both extract copies.
    act_ops = []  # ("sq"|"cp", u, i)
    if ileave:
        for base in range(0, nu, 2):
            pair = [base] + ([base + 1] if base + 1 < nu else [])
            for i in range(K):
                for u in pair:
                    act_ops.append(("sq", u, i))
            for u in pair:
                act_ops.append(("cp", u, 0))
    else:
        for u in range(nu):
            for i in range(K):
                act_ops.append(("sq", u, i))
            act_ops.append(("cp", u, 0))
    _sq_pos = {(u, i): n + 1 for n, (kind, u, i) in enumerate(act_ops)
               if kind == "sq"}
    _cp_pos = {u: n + 1 for n, (kind, u, i) in enumerate(act_ops)
               if kind == "cp"}

    def act_through_square(u, i):
        return _sq_pos[(u, i)]

    def act_through_copy(u):
        return _cp_pos[u]

    def dma_through_out(u):  # X1 init DMA then one out-DMA per unit
        return 16 * (2 + u)

    with ExitStack() as ctx:
        sb = lambda shape, name, dt: ctx.enter_context(
            nc.sbuf_tensor(name, shape, dt))
        X = [sb([128, L], f"Xt{cc}", F32) for cc in range(2)]
        negK = sb([128, K], "negKt", F32)
        m0 = sb([128, TFREE], "m0t", SDT)
        S = [[sb([128, TFREE], f"St{s}_{i}", SDT) for i in range(2)]
             for s in range(SLOTS)]
        M = [sb([128, TFREE], f"Mt{s}", SDT) for s in range(SLOTS)]
        D = [[sb([128, TFREE], f"Dt{s}_{i}", SDT) for i in range(2)]
             for s in range(SLOTS)]
        OB = [sb([128, NWC], f"OBt{s}", F32) for s in range(SLOTS)]

        dma_sem = ctx.enter_context(nc.semaphore("dma_sem"))
        dma0_sem = ctx.enter_context(nc.semaphore("dma0_sem"))
        act_sem = ctx.enter_context(nc.semaphore("act_sem"))
        dve_sem = ctx.enter_context(nc.semaphore("dve_sem"))
        gp_sem = ctx.enter_context(nc.semaphore("gp_sem"))
        block = ctx.enter_context(nc.Block())

        @block.sync
        def _(sync):
            # negK + X0 first so cc0 compute starts before X1 lands.
            # X[cc] partition p holds x[4*cc + p//32, :] (source AP
            # replicates each channel row 32x via a step-0 dim)
            ksrc = bass.AP(k_d.tensor, 0, [[0, 4], [K, F], [1, K]])
            sync.dma_start(negK.ap(), ksrc).then_inc(dma0_sem, 16)
            for cc in range(2):
                src = bass.AP(x_d.tensor, 4 * cc * L,
                              [[L, 4], [0, 32], [1, L]])
                sync.dma_start(X[cc].ap(), src).then_inc(
                    dma0_sem if cc == 0 else dma_sem, 16)
            for u, (cc, wc) in enumerate(UNITS_R):
                s = u % SLOTS
                sync.wait_ge(act_sem, act_through_copy(u))
                sync.dma_start(
                    out_d[128 * cc:128 * (cc + 1),
                          NWC * wc:NWC * (wc + 1)],
                    OB[s].ap()).then_inc(dma_sem, 16)

        def emit_m(eng, u, s, prev):
            if small_m:
                return eng.tensor_tensor(M[s].ap()[:, 1:5], prev[:, 1:5],
                                         prev[:, :4],
                                         mybir.AluOpType.min)
            return eng.tensor_tensor(M[s].ap()[:, 1:], prev[:, 1:],
                                     prev[:, :-1], mybir.AluOpType.min)

        if gp_m:
            @block.gpsimd
            def _(gpsimd):
                dve_waited = 0
                for u, (cc, wc) in enumerate(UNITS_R):
                    s = u % SLOTS
                    for i in range(1, K):
                        need = dve_through_scan(u, i - 1)
                        if need > dve_waited:
                            gpsimd.wait_ge(dve_sem, need)
                            dve_waited = need
                        emit_m(gpsimd, u, s,
                               S[s][(i - 1) % 2].ap()).then_inc(gp_sem, 1)

        @block.vector
        def _(vector):
            # init: m0 = BIG with 0 at each segment's cell j=0 (offset 1);
            # M BIG (so m[0] defined); D separator columns BIG
            vector.memset(m0.ap(), big).then_inc(dve_sem, 1)
            m0_seg = m0.ap().rearrange("p (w s) -> p w s", s=SEG)
            vector.memset(m0_seg[:, :, 1], 0.0).then_inc(dve_sem, 1)
            for s in range(SLOTS):
                vector.memset(M[s].ap(), big).then_inc(dve_sem, 1)
                for i in range(2):
                    d_seg = D[s][i].ap().rearrange("p (w s) -> p w s", s=SEG)
                    vector.memset(d_seg[:, :, 0], big).then_inc(dve_sem, 1)
            act_waited = 0
            gp_waited = 0
            for kind, u, i in dve_ops:
                s = u % SLOTS
                if kind == "m":
                    if gp_m:
                        continue
                    emit_m(vector, u, s,
                           S[s][(i - 1) % 2].ap()).then_inc(dve_sem, 1)
                    continue
                # scan: row i reads m0 (i=0) or M[s], writes S[s][i%2]
                if i == 0:
                    m_ap = m0.ap()
                else:
                    if gp_m:
                        need = gp_through_m(u, i)
                        if need > gp_waited:
                            vector.wait_ge(gp_sem, need)
                            gp_waited = need
                    m_ap = M[s].ap()
                need = act_through_square(u, i)
                if need > act_waited:
                    vector.wait_ge(act_sem, need)
                    act_waited = need
                if small_scan:
                    vector.tensor_tensor_scan(
                        S[s][i % 2].ap()[:, :4], m_ap[:, :4],
                        D[s][i % 2].ap()[:, :4], float(big),
                        op0=mybir.AluOpType.min,
                        op1=mybir.AluOpType.add).then_inc(dve_sem, 1)
                else:
                    vector.tensor_tensor_scan(
                        S[s][i % 2].ap(), m_ap, D[s][i % 2].ap(),
                        float(big),
                        op0=mybir.AluOpType.min,
                        op1=mybir.AluOpType.add).then_inc(dve_sem, 1)

        @block.scalar
        def _(scalar):
            scalar.wait_ge(dma0_sem, 32)  # negK + X0
            dve_waited = 0
            dma_waited = 0
            x1_waited = False
            for kind, u, i in act_ops:
                cc, wc = UNITS_R[u]
                s = u % SLOTS
                if cc == 1 and not x1_waited:
                    scalar.wait_ge(dma_sem, 16)  # X1
                    x1_waited = True
                if kind == "sq":
                    xt = X[cc].ap()
                    win = bass.AP(xt.tensor, xt.offset + 5 * NWC * wc,
                                  [list(xt.ap[0]), [5, NWC], [1, PROC]])
                    # WAR: D[s][i%2] was last read by an earlier scan
                    if i >= 2:
                        need = dve_through_scan(u, i - 2)
                    elif u >= SLOTS:
                        need = dve_through_scan(u - SLOTS, 8 + i)
                    else:
                        need = 0
                    if need > dve_waited:
                        scalar.wait_ge(dve_sem, need)
                        dve_waited = need
                    d_seg = D[s][i % 2].ap().rearrange(
                        "p (w s) -> p w s", s=SEG)
                    if small_act:
                        scalar.activation(
                            d_seg[:, :1, 1:], win[:, :1, :],
                            mybir.ActivationFunctionType.Square,
                            bias=negK.ap()[:, i:i + 1],
                            scale=1.0).then_inc(act_sem, 1)
                    else:
                        scalar.activation(
                            d_seg[:, :, 1:], win,
                            mybir.ActivationFunctionType.Square,
                            bias=negK.ap()[:, i:i + 1],
                            scale=1.0).then_inc(act_sem, 1)
                else:
                    # extract: cell j=19 lives at segment offset 20; final
                    # row (i=9, odd) lands in S[s][1]
                    need = dve_through_scan(u, K - 1)
                    if need > dve_waited:
                        scalar.wait_ge(dve_sem, need)
                        dve_waited = need
                    if u >= SLOTS:
                        dneed = dma_through_out(u - SLOTS)
                        if dneed > dma_waited:
                            scalar.wait_ge(dma_sem, dneed)
                            dma_waited = dneed
                    s_seg = S[s][1].ap().rearrange("p (w s) -> p w s",
                                                   s=SEG)
                    scalar.copy(OB[s].ap(), s_seg[:, :, SEG - 1]).then_inc(
                        act_sem, 1)
    return nc


_NC_CACHE = None


def kernel(x: np.ndarray, kernels: np.ndarray) -> np.ndarray:
    global _NC_CACHE
    if _NC_CACHE is None:
        _NC_CACHE = _build_nc()
    nc = _NC_CACHE
    x = np.ascontiguousarray(x, dtype=np.float32)
    negk = np.ascontiguousarray(-np.asarray(kernels, dtype=np.float32))
    in_maps = [{"x": x[b], "negk": negk} for b in range(B)]
    res = run_bass_kernel_spmd(nc, in_maps, core_ids=list(range(B)))
    out = np.stack([res.results[b]["out"] for b in range(B)], axis=0)
    return out[:, :, :NW]
